# revision 1
# baseline (speedup 1.0000x reference)
"""Trainium2 Bass kernel for nn_CrossVariableMixingHydra.

Math (per batch b):
  h      = x[b].T                         # (C, L) view; we keep x in native (L, C) layout
  h_low  = h @ Wd.T + bd                  # (C, R)
  Q,K    = l2norm(h_low @ W.T) ; V,G = h_low @ W.T (+bg)
  gf     = sum_c K*V                      # (R,)
  h_attn = Q * gf * sigmoid(G)
  h_mix  = h_attn @ Wu.T + bu             # (C, L)
  out    = LayerNorm_C(x + alpha*h_mix.T) * gamma + beta

Strategy: data-parallel over batch on 8 cores (8 batches/core). On each core the
whole computation is fused in one program; x is loaded once per batch and used
both as the down-projection matmul operand (float32r) and the fp32 residual.
The attention branch runs in bf16/f32r (its contribution to the output is
~0.35% of the residual, so reduced precision there is invisible); the residual
add and LayerNorm are exact fp32 on the vector/scalar/pool engines.
"""

import numpy as np
import ml_dtypes

import concourse.bass as bass
import concourse.mybir as mybir
import concourse.tile as tile
import concourse.bass_utils as bass_utils

B, L, C, R = 64, 2048, 512, 64
N_CORES = 8
BPC = B // N_CORES          # batches per core
NCH = L // 128              # l-chunks per batch
EPS_LN = 1e-5
EPS_NORM_SQ = 1e-24         # (1e-12)^2 : sqrt(ssq + eps^2) == max(sqrt(ssq), 1e-12)

f32 = mybir.dt.float32
f32r = mybir.dt.float32r
bf16 = mybir.dt.bfloat16
Alu = mybir.AluOpType
Act = mybir.ActivationFunctionType


def _split_waits(nc, max_waits: int = 1):
    """This container's walrus build rejects instructions carrying more than
    one sync wait. Move excess waits onto preceding NoOps on the same engine
    (engines execute in order, so semantics are unchanged)."""
    for f in nc.m.functions:
        for bb in f.blocks:
            insts = bb.instructions
            i = 0
            while i < len(insts):
                inst = insts[i]
                si = inst.sync_info
                if si is not None and si.on_wait and len(si.on_wait) > max_waits:
                    waits = list(si.on_wait)
                    si.on_wait = waits[:max_waits]
                    extra = waits[max_waits:]
                    nops = []
                    for j in range(0, len(extra), max_waits):
                        nop = mybir.InstNoOp(name=f"{inst.name}-ws{j}", ins=[], outs=[])
                        nop.engine = inst.engine
                        nop.sync_info = mybir.SyncInfo(
                            on_wait=extra[j : j + max_waits], on_update=[]
                        )
                        nops.append(nop)
                    for k, nop in enumerate(nops):
                        insts.insert(i + k, nop)
                    i += len(nops)
                i += 1
    return nc


def build(trivial_affine: bool, repeat: int = 1):
    nc = bass.Bass("TRN2", target_bir_lowering=False, debug=False, num_devices=1)

    x_ap = nc.dram_tensor("x", [BPC, L, C], f32r, kind="ExternalInput").ap()
    wdt_ap = nc.dram_tensor("wdt", [L, R], f32r, kind="ExternalInput").ap()
    bd_ap = nc.dram_tensor("bdc", [R, 1], f32, kind="ExternalInput").ap()
    wqk_ap = nc.dram_tensor("wqk", [R + 1, 128], bf16, kind="ExternalInput").ap()
    wvg_ap = nc.dram_tensor("wvg", [R + 1, 128], bf16, kind="ExternalInput").ap()
    wut_ap = nc.dram_tensor("wut", [R + 1, L], bf16, kind="ExternalInput").ap()
    selm_ap = nc.dram_tensor("selm", [128, 2], f32, kind="ExternalInput").ap()
    epsn_ap = nc.dram_tensor("epsn", [2, 1], f32, kind="ExternalInput").ap()
    epsl_ap = nc.dram_tensor("epsl", [128, 1], f32, kind="ExternalInput").ap()
    sel2_ap = nc.dram_tensor("sel2", [2, 128], f32, kind="ExternalInput").ap()
    if not trivial_affine:
        gam_ap = nc.dram_tensor("gamb", [128, C], f32, kind="ExternalInput").ap()
        bet_ap = nc.dram_tensor("betb", [128, C], f32, kind="ExternalInput").ap()
    out_ap = nc.dram_tensor("out", [BPC, L, C], f32, kind="ExternalOutput").ap()

    with tile.TileContext(nc) as tc:
        with (
            tc.tile_pool(name="consts", bufs=1) as cp,
            tc.tile_pool(name="xp", bufs=2) as xp,
            tc.tile_pool(name="attn", bufs=2) as ap_,
            tc.tile_pool(name="vs", bufs=NCH + 2) as vp,
            tc.tile_pool(name="ys", bufs=3) as yp,
            tc.tile_pool(name="sq", bufs=2) as qp,
            tc.tile_pool(name="st", bufs=2) as sp,
            tc.tile_pool(name="ps_hl", bufs=1, space="PSUM") as ps_hl,
            tc.tile_pool(name="ps_qk", bufs=1, space="PSUM") as ps_qk,
            tc.tile_pool(name="ps_vg", bufs=1, space="PSUM") as ps_vg,
            tc.tile_pool(name="ps_sq", bufs=1, space="PSUM") as ps_sq,
            tc.tile_pool(name="ps_bc", bufs=1, space="PSUM") as ps_bc,
            tc.tile_pool(name="ps_up", bufs=3, space="PSUM") as ps_up,
        ):
            # --- constants (loaded once) ---
            wdt = cp.tile([128, NCH * R], f32r)
            nc.sync.dma_start(
                wdt[:].rearrange("p (n r) -> p n r", n=NCH),
                wdt_ap[:].rearrange("(n p) r -> p n r", p=128),
            )
            bd = cp.tile([R, 1], f32)
            nc.sync.dma_start(bd[:], bd_ap[:])
            wqk = cp.tile([R + 1, 128], bf16)
            nc.sync.dma_start(wqk[:], wqk_ap[:])
            wvg = cp.tile([R + 1, 128], bf16)
            nc.sync.dma_start(wvg[:], wvg_ap[:])
            wut = cp.tile([R + 1, L], bf16)
            nc.sync.dma_start(wut[:], wut_ap[:])
            selm = cp.tile([128, 2], f32)
            nc.sync.dma_start(selm[:], selm_ap[:])
            sel2 = cp.tile([2, 128], f32)
            nc.sync.dma_start(sel2[:], sel2_ap[:])
            epsn = cp.tile([2, 1], f32)
            nc.sync.dma_start(epsn[:], epsn_ap[:])
            epsl = cp.tile([128, 1], f32)
            nc.sync.dma_start(epsl[:], epsl_ap[:])
            if not trivial_affine:
                gam = cp.tile([128, C], f32)
                nc.sync.dma_start(gam[:], gam_ap[:])
                bet = cp.tile([128, C], f32)
                nc.sync.dma_start(bet[:], bet_ap[:])

            def batch_body(b):
                # --- load x[b] as 16 chunks of (128 l, 512 c) ---
                xb = xp.tile([128, NCH * C], f32r, tag="xb")
                GRP = 4
                for g in range(NCH // GRP):
                    nc.sync.dma_start(
                        xb[:, g * GRP * C : (g + 1) * GRP * C].rearrange(
                            "p (n c) -> p n c", n=GRP
                        ),
                        x_ap[b, g * GRP * 128 : (g + 1) * GRP * 128, :].rearrange(
                            "(n p) c -> p n c", p=128
                        ),
                    )

                # --- down-projection: h_lowT (R, C) = sum_l Wd.T[l,r] x[l,c] ---
                hl_ps = ps_hl.tile([R, C], f32, tag="hl")
                for k in range(NCH):
                    nc.tensor.matmul(
                        hl_ps[:],
                        wdt[:, k * R : (k + 1) * R],
                        xb[:, k * C : (k + 1) * C],
                        start=(k == 0),
                        stop=(k == NCH - 1),
                    )
                # + bd, round to bf16, append ones row for bias tricks
                hle = ap_.tile([R + 1, C], bf16, tag="hle")
                nc.scalar.activation(hle[0:R, :], hl_ps[:], Act.Identity, bias=bd[:], scale=1.0)
                nc.gpsimd.memset(hle[R : R + 1, :], 1.0)

                # --- QKT / VGT : (128, C) each; rows 0-63 = Q^T / V^T, 64-127 = K^T / G^T+bg ---
                qk_ps = ps_qk.tile([128, C], f32, tag="qk")
                nc.tensor.matmul(qk_ps[:], wqk[:], hle[:], start=True, stop=True)
                vg_ps = ps_vg.tile([128, C], f32, tag="vg")
                nc.tensor.matmul(vg_ps[:], wvg[:], hle[:], start=True, stop=True)

                qk_s = ap_.tile([128, C], f32, tag="qks")
                nc.scalar.copy(qk_s[:], qk_ps[:])
                gate = ap_.tile([R, C], f32, tag="gate")
                nc.scalar.activation(gate[:], vg_ps[R:128, :], Act.Sigmoid)

                # --- l2 norms: ssq rows via PE partition-reduce, rsqrt, broadcast ---
                qk2 = ap_.tile([128, C], f32, tag="qk2")
                nc.vector.scalar_tensor_tensor(
                    qk2[:], qk_ps[:], 1.0, qk_s[:], op0=Alu.mult, op1=Alu.mult
                )
                ssq_ps = ps_sq.tile([2, C], f32, tag="ssq")
                nc.tensor.matmul(ssq_ps[:], selm[:], qk2[:], start=True, stop=True)
                nrm = sp.tile([2, C], f32, tag="nrm")
                nc.scalar.activation(nrm[:], ssq_ps[:], Act.Sqrt, bias=epsn[:])
                rstd2 = sp.tile([2, C], f32, tag="rstd2")
                nc.vector.reciprocal(rstd2[:], nrm[:])
                bc_ps = ps_bc.tile([128, C], f32, tag="bc")
                nc.tensor.matmul(bc_ps[:], sel2[:], rstd2[:], start=True, stop=True)

                # --- global feature gf[s] = sum_c rstdk[c] K^T[s,c] V^T[s,c] ---
                kv = ap_.tile([R, C], f32, tag="kv")
                nc.vector.scalar_tensor_tensor(
                    kv[:], vg_ps[0:R, :], 1.0, qk_s[R:128, :], op0=Alu.mult, op1=Alu.mult
                )
                gf = sp.tile([R, 1], f32, tag="gf")
                gfs = ap_.tile([R, C], f32, tag="gfs")
                nc.vector.scalar_tensor_tensor(
                    gfs[:], bc_ps[R:128, :], 1.0, kv[:], op0=Alu.mult, op1=Alu.mult,
                    accum_out=gf[:],
                )

                # --- h_attn^T = (rstdq_bc * gf) * Q^T * gate ---
                at = ap_.tile([R, C], f32, tag="at")
                nc.vector.scalar_tensor_tensor(
                    at[:], bc_ps[0:R, :], gf[:], qk_s[0:R, :], op0=Alu.mult, op1=Alu.mult
                )
                ha = ap_.tile([R + 1, C], bf16, tag="ha")
                nc.vector.tensor_tensor(ha[0:R, :], at[:], gate[:], op=Alu.mult)
                nc.gpsimd.memset(ha[R : R + 1, :], 1.0)

                # --- upmix + residual + LN stats per l-chunk ---
                sums = sp.tile([128, NCH], f32, tag="sums")
                sums2 = sp.tile([128, NCH], f32, tag="sums2")
                vts = []
                for k in range(NCH):
                    up_ps = ps_up.tile([128, C], f32, tag="up")
                    nc.tensor.matmul(
                        up_ps[:], wut[:, k * 128 : (k + 1) * 128], ha[:],
                        start=True, stop=True,
                    )
                    vt = vp.tile([128, C], f32, tag="v")
                    nc.vector.scalar_tensor_tensor(
                        vt[:], up_ps[:], 1.0,
                        xb[:, k * C : (k + 1) * C].bitcast(f32),
                        op0=Alu.mult, op1=Alu.add,
                        accum_out=sums[:, k : k + 1],
                    )
                    sqs = qp.tile([128, C], f32, tag="sqs")
                    nc.scalar.activation(
                        sqs[:], vt[:], Act.Square, accum_out=sums2[:, k : k + 1]
                    )
                    vts.append(vt)

                # --- batched LN stats: var = E[v^2] - mu^2, rstd, -mu*rstd ---
                mu = sp.tile([128, NCH], f32, tag="mu")
                nc.vector.tensor_scalar_mul(mu[:], sums[:], 1.0 / C)
                nmu2 = sp.tile([128, NCH], f32, tag="nmu2")
                nc.vector.scalar_tensor_tensor(
                    nmu2[:], mu[:], -1.0, mu[:], op0=Alu.mult, op1=Alu.mult
                )
                var = sp.tile([128, NCH], f32, tag="var")
                nc.vector.scalar_tensor_tensor(
                    var[:], sums2[:], 1.0 / C, nmu2[:], op0=Alu.mult, op1=Alu.add
                )
                sd = sp.tile([128, NCH], f32, tag="sd")
                nc.scalar.activation(sd[:], var[:], Act.Sqrt, bias=epsl[:])
                r0 = sp.tile([128, NCH], f32, tag="r0")
                nc.vector.reciprocal(r0[:], sd[:])
                # ACT Sqrt is table-approximated (~2e-4); one Newton step on
                # rsqrt: r1 = r0 * (1.5 - 0.5 * (var+eps) * r0^2)
                vpe = sp.tile([128, NCH], f32, tag="vpe")
                nc.vector.tensor_scalar(vpe[:], var[:], 1.0, epsl[:], op0=Alu.mult, op1=Alu.add)
                t1 = sp.tile([128, NCH], f32, tag="t1")
                nc.vector.tensor_tensor(t1[:], vpe[:], r0[:], op=Alu.mult)
                t2 = sp.tile([128, NCH], f32, tag="t2")
                nc.vector.tensor_tensor(t2[:], t1[:], r0[:], op=Alu.mult)
                t3 = sp.tile([128, NCH], f32, tag="t3")
                nc.vector.tensor_scalar(t3[:], t2[:], -0.5, 1.5, op0=Alu.mult, op1=Alu.add)
                rstd = sp.tile([128, NCH], f32, tag="rstd")
                nc.vector.tensor_tensor(rstd[:], r0[:], t3[:], op=Alu.mult)
                nmr = sp.tile([128, NCH], f32, tag="nmr")
                nc.vector.scalar_tensor_tensor(
                    nmr[:], mu[:], -1.0, rstd[:], op0=Alu.mult, op1=Alu.mult
                )

                # --- finalize y = (v - mu) * rstd [* gamma + beta] and store ---
                GRP = 4
                for g in range(NCH // GRP):
                    yg = yp.tile([128, GRP * C], f32, tag="y")
                    for j in range(GRP):
                        k = g * GRP + j
                        ys = yg[:, j * C : (j + 1) * C]
                        nc.gpsimd.tensor_scalar(
                            ys, vts[k][:], rstd[:, k : k + 1], nmr[:, k : k + 1],
                            op0=Alu.mult, op1=Alu.add,
                        )
                        if not trivial_affine:
                            nc.vector.tensor_tensor(ys, ys, gam[:], op=Alu.mult)
                            nc.gpsimd.tensor_tensor(ys, ys, bet[:], op=Alu.add)
                    nc.sync.dma_start(
                        out_ap[b, g * GRP * 128 : (g + 1) * GRP * 128, :].rearrange(
                            "(n p) c -> p n c", p=128
                        ),
                        yg[:].rearrange("p (n c) -> p n c", n=GRP),
                    )

            if repeat == 1:
                for b in range(BPC):
                    batch_body(b)
            else:
                with tc.For_i(0, repeat, 1):
                    for b in range(BPC):
                        batch_body(b)

    return _split_waits(nc)


def prep_inputs(x, Wd, bd, Wq, Wk, Wv, Wg, bg, Wu, bu, gamma, beta, alpha):
    x = np.ascontiguousarray(np.asarray(x, dtype=np.float32))
    Wd = np.asarray(Wd, np.float32); bd = np.asarray(bd, np.float32)
    Wq = np.asarray(Wq, np.float32); Wk = np.asarray(Wk, np.float32)
    Wv = np.asarray(Wv, np.float32); Wg = np.asarray(Wg, np.float32)
    bg = np.asarray(bg, np.float32)
    Wu = np.asarray(Wu, np.float32); bu = np.asarray(bu, np.float32)
    gamma = np.asarray(gamma, np.float32); beta = np.asarray(beta, np.float32)
    alpha = np.float32(np.asarray(alpha))

    trivial = bool(np.all(gamma == 1.0) and np.all(beta == 0.0))

    wdt = np.ascontiguousarray(Wd.T)                       # (L, R) f32
    bdc = np.ascontiguousarray(bd[:, None])                # (R, 1)
    wqk = np.zeros((R + 1, 128), np.float32)
    wqk[0:R, 0:R] = Wq.T
    wqk[0:R, R:128] = Wk.T
    wvg = np.zeros((R + 1, 128), np.float32)
    wvg[0:R, 0:R] = Wv.T
    wvg[0:R, R:128] = Wg.T
    wvg[R, R:128] = bg
    wut = np.zeros((R + 1, L), np.float32)
    wut[0:R, :] = alpha * Wu.T
    wut[R, :] = alpha * bu
    selm = np.zeros((128, 2), np.float32)
    selm[0:R, 0] = 1.0
    selm[R:128, 1] = 1.0
    sel2 = np.zeros((2, 128), np.float32)
    sel2[0, 0:R] = 1.0
    sel2[1, R:128] = 1.0

    common = dict(
        wdt=wdt,
        bdc=bdc,
        wqk=wqk.astype(ml_dtypes.bfloat16),
        wvg=wvg.astype(ml_dtypes.bfloat16),
        wut=wut.astype(ml_dtypes.bfloat16),
        selm=selm,
        sel2=sel2,
        epsn=np.full((2, 1), EPS_NORM_SQ, np.float32),
        epsl=np.full((128, 1), EPS_LN, np.float32),
    )
    if not trivial:
        common["gamb"] = np.ascontiguousarray(np.tile(gamma[None, :], (128, 1)))
        common["betb"] = np.ascontiguousarray(np.tile(beta[None, :], (128, 1)))

    in_maps = []
    for c in range(N_CORES):
        m = dict(common)
        m["x"] = np.ascontiguousarray(x[c * BPC : (c + 1) * BPC])
        in_maps.append(m)
    return in_maps, trivial


_nc_cache = {}


def kernel(**inputs) -> np.ndarray:
    in_maps, trivial = prep_inputs(**inputs)
    if trivial not in _nc_cache:
        _nc_cache[trivial] = build(trivial)
    nc = _nc_cache[trivial]
    res = bass_utils.run_bass_kernel_spmd(nc, in_maps, core_ids=list(range(N_CORES)))
    out = np.concatenate([res.results[c]["out"] for c in range(N_CORES)], axis=0)
    return out.astype(np.float32, copy=False)



# revision 17
# speedup vs baseline: 1.1422x; 1.1422x over previous
"""Trainium2 Bass kernel for nn_CrossVariableMixingHydra.

Math (per batch b):
  h      = x[b].T                         # (C, L) view; we keep x in native (L, C) layout
  h_low  = h @ Wd.T + bd                  # (C, R)
  Q,K    = l2norm(h_low @ W.T) ; V,G = h_low @ W.T (+bg)
  gf     = sum_c K*V                      # (R,)
  h_attn = Q * gf * sigmoid(G)
  h_mix  = h_attn @ Wu.T + bu             # (C, L)
  out    = LayerNorm_C(x + alpha*h_mix.T) * gamma + beta

Strategy: data-parallel over batch on 8 cores (8 batches/core). Memory-bound
problem, so HBM traffic is halved by shipping x as bf16 and storing the
output as bf16 (host casts back to f32); the rel-err budget (2e-2) dwarfs
bf16 rounding (~2e-3). Rows are paired per partition (l = n*256 + 2p + j)
so bf16 DMA descriptors stay 2KB. The residual add x + alpha*h_mix runs on
the PE as an extra identity matmul accumulating into the upmix PSUM bank,
which removes a full DVE elementwise stream. LayerNorm stats are computed
per group of 4 l-chunks so stores start early and overlap compute.
"""

import numpy as np
import ml_dtypes

import concourse.bass as bass
import concourse.mybir as mybir
import concourse.tile as tile
import concourse.bass_utils as bass_utils

B, L, C, R = 64, 2048, 512, 64
N_CORES = 8
BPC = B // N_CORES          # batches per core
NCH = L // 128              # l-chunks per batch (16)
NGRP = NCH // 4             # stats/store groups per batch (4)
EPS_LN = 1e-5
EPS_NORM_SQ = 1e-24         # (1e-12)^2 : sqrt(ssq + eps^2) == max(sqrt(ssq), 1e-12)

f32 = mybir.dt.float32
bf16 = mybir.dt.bfloat16
Alu = mybir.AluOpType
Act = mybir.ActivationFunctionType

# engine assignment knobs, keyed by chunk-index-within-half (0..7).
# Streams: vt (PSUM->SBUF v materialize + row-sum), sq (v^2 + row-sum),
# ys (final (v-mu)*rstd). Defaults balance ACT/DVE/POOL under the cost model.
VT_DVE_J = {3, 7}            # vt on DVE (rest ACT)
SQ_POOL_J = set()            # sq on POOL gpsimd — unsupported (walrus engine check)
YS_ACT_J = {0}               # ys on ACT
YS_DVE_J = {2, 4, 6}         # ys on DVE (rest POOL)


def _split_waits(nc, max_waits: int = 1):
    """This container's walrus build rejects instructions carrying more than
    one sync wait. Move excess waits onto preceding NoOps on the same engine
    (engines execute in order, so semantics are unchanged)."""
    for f in nc.m.functions:
        for bb in f.blocks:
            insts = bb.instructions
            i = 0
            while i < len(insts):
                inst = insts[i]
                si = inst.sync_info
                if si is not None and si.on_wait and len(si.on_wait) > max_waits:
                    waits = list(si.on_wait)
                    si.on_wait = waits[:max_waits]
                    extra = waits[max_waits:]
                    nops = []
                    for j in range(0, len(extra), max_waits):
                        nop = mybir.InstNoOp(name=f"{inst.name}-ws{j}", ins=[], outs=[])
                        nop.engine = inst.engine
                        nop.sync_info = mybir.SyncInfo(
                            on_wait=extra[j : j + max_waits], on_update=[]
                        )
                        nops.append(nop)
                    for k, nop in enumerate(nops):
                        insts.insert(i + k, nop)
                    i += len(nops)
                i += 1
    return nc


def build(trivial_affine: bool, repeat: int = 1):
    nc = bass.Bass("TRN2", target_bir_lowering=False, debug=False, num_devices=1)

    x_ap = nc.dram_tensor("x", [BPC, L, C], bf16, kind="ExternalInput").ap()
    wdt_ap = nc.dram_tensor("wdt", [128, NCH * R], bf16, kind="ExternalInput").ap()
    bd_ap = nc.dram_tensor("bdc", [R, 1], f32, kind="ExternalInput").ap()
    wqk_ap = nc.dram_tensor("wqk", [R + 1, 128], bf16, kind="ExternalInput").ap()
    wvg_ap = nc.dram_tensor("wvg", [R + 1, 128], bf16, kind="ExternalInput").ap()
    wut_ap = nc.dram_tensor("wut", [R + 1, NCH * 128], bf16, kind="ExternalInput").ap()
    id_ap = nc.dram_tensor("id128", [128, 128], bf16, kind="ExternalInput").ap()
    selm_ap = nc.dram_tensor("selm", [128, 2], bf16, kind="ExternalInput").ap()
    sel2_ap = nc.dram_tensor("sel2", [2, 128], bf16, kind="ExternalInput").ap()
    epsn_ap = nc.dram_tensor("epsn", [2, 1], f32, kind="ExternalInput").ap()
    epsl_ap = nc.dram_tensor("epsl", [128, 1], f32, kind="ExternalInput").ap()
    if not trivial_affine:
        gam_ap = nc.dram_tensor("gamb", [128, C], f32, kind="ExternalInput").ap()
        bet_ap = nc.dram_tensor("betb", [128, C], f32, kind="ExternalInput").ap()
    out_ap = nc.dram_tensor("out", [BPC, L, C], bf16, kind="ExternalOutput").ap()

    with tile.TileContext(nc) as tc:
        with (
            tc.tile_pool(name="consts", bufs=1) as cp,
            tc.tile_pool(name="xp", bufs=3) as xp,
            tc.tile_pool(name="attn", bufs=2) as ap_,
            tc.tile_pool(name="vs", bufs=12) as vp,
            tc.tile_pool(name="sqp", bufs=4) as sqp,
            tc.tile_pool(name="ys", bufs=2) as yp,
            tc.tile_pool(name="st", bufs=2) as sp,
            tc.tile_pool(name="ps_hl", bufs=1, space="PSUM") as ps_hl,
            tc.tile_pool(name="ps_qk", bufs=1, space="PSUM") as ps_qk,
            tc.tile_pool(name="ps_vg", bufs=1, space="PSUM") as ps_vg,
            tc.tile_pool(name="ps_sb", bufs=1, space="PSUM") as ps_sb,
            tc.tile_pool(name="ps_up", bufs=3, space="PSUM") as ps_up,
        ):
            # --- constants (loaded once) ---
            wdt = cp.tile([128, NCH * R], bf16)
            nc.sync.dma_start(wdt[:], wdt_ap[:])
            bd = cp.tile([R, 1], f32)
            nc.sync.dma_start(bd[:], bd_ap[:])
            wqk = cp.tile([R + 1, 128], bf16)
            nc.sync.dma_start(wqk[:], wqk_ap[:])
            wvg = cp.tile([R + 1, 128], bf16)
            nc.sync.dma_start(wvg[:], wvg_ap[:])
            wut = cp.tile([R + 1, NCH * 128], bf16)
            nc.sync.dma_start(wut[:], wut_ap[:])
            id128 = cp.tile([128, 128], bf16)
            nc.sync.dma_start(id128[:], id_ap[:])
            selm = cp.tile([128, 2], bf16)
            nc.sync.dma_start(selm[:], selm_ap[:])
            sel2 = cp.tile([2, 128], bf16)
            nc.sync.dma_start(sel2[:], sel2_ap[:])
            epsn = cp.tile([2, 1], f32)
            nc.sync.dma_start(epsn[:], epsn_ap[:])
            epsl = cp.tile([128, 1], f32)
            nc.sync.dma_start(epsl[:], epsl_ap[:])
            if not trivial_affine:
                gam = cp.tile([128, C], f32)
                nc.sync.dma_start(gam[:], gam_ap[:])
                bet = cp.tile([128, C], f32)
                nc.sync.dma_start(bet[:], bet_ap[:])

            def load_x(b):
                # one 2MB DMA; per-partition descriptors are 2KB (j,c
                # contiguous: rows 2p and 2p+1 of each 256-row block)
                xb = xp.tile([128, NCH * C], bf16, tag="xb")
                nc.sync.dma_start(
                    xb[:].rearrange("p (n j c) -> p n j c", n=NCH // 2, j=2),
                    x_ap[b].rearrange("(n p j) c -> p n j c", p=128, j=2),
                )
                return xb

            def attn_phase(b, xb):
                # --- down-projection: h_lowT (R, C) = sum_l Wd.T[l,r] x[l,c] ---
                hl_ps = ps_hl.tile([R, C], f32, tag="hl")
                for k in range(NCH):
                    nc.tensor.matmul(
                        hl_ps[:],
                        wdt[:, k * R : (k + 1) * R],
                        xb[:, k * C : (k + 1) * C],
                        start=(k == 0),
                        stop=(k == NCH - 1),
                    )
                # + bd, round to bf16, append ones row for bias tricks
                hle = ap_.tile([R + 1, C], bf16, tag="hle")
                nc.scalar.activation(hle[0:R, :], hl_ps[:], Act.Identity, bias=bd[:], scale=1.0)
                nc.gpsimd.memset(hle[R : R + 1, :], 1.0)

                # --- QKT / VGT : (128, C) each; rows 0-63 = Q^T / V^T, 64-127 = K^T / G^T+bg ---
                qk_ps = ps_qk.tile([128, C], f32, tag="qk")
                nc.tensor.matmul(qk_ps[:], wqk[:], hle[:], start=True, stop=True)
                vg_ps = ps_vg.tile([128, C], f32, tag="vg")
                nc.tensor.matmul(vg_ps[:], wvg[:], hle[:], start=True, stop=True)

                qk_s = ap_.tile([128, C], f32, tag="qks")
                nc.scalar.copy(qk_s[:], qk_ps[:])
                qk2 = ap_.tile([128, C], bf16, tag="qk2")
                nc.scalar.activation(qk2[:], qk_ps[:], Act.Square)
                gate = ap_.tile([R, C], bf16, tag="gate")
                nc.scalar.activation(gate[:], vg_ps[R:128, :], Act.Sigmoid)

                # --- l2 norms: ssq rows via PE partition-reduce, sqrt, recip, broadcast ---
                ssq_ps = ps_sb.tile([2, C], f32, tag="ssq")
                nc.tensor.matmul(ssq_ps[:], selm[:], qk2[:], start=True, stop=True)
                nrm = sp.tile([2, C], f32, tag="nrm")
                nc.scalar.activation(nrm[:], ssq_ps[:], Act.Sqrt, bias=epsn[:])
                rstd2 = sp.tile([2, C], bf16, tag="rstd2")
                with nc.allow_low_precision(reason="norm scale feeds bf16 matmul"):
                    nc.vector.reciprocal(rstd2[:], nrm[:])
                bc_ps = ps_sb.tile([128, C], f32, tag="bc")
                nc.tensor.matmul(bc_ps[:], sel2[:], rstd2[:], start=True, stop=True)

                # --- global feature gf[s] = sum_c rstdk[c] K^T[s,c] V^T[s,c] ---
                kv = ap_.tile([R, C], f32, tag="kv")
                nc.vector.scalar_tensor_tensor(
                    kv[:], vg_ps[0:R, :], 1.0, qk_s[R:128, :], op0=Alu.mult, op1=Alu.mult
                )
                gf = sp.tile([R, 1], f32, tag="gf")
                gfs = ap_.tile([R, C], f32, tag="gfs")
                nc.vector.scalar_tensor_tensor(
                    gfs[:], bc_ps[R:128, :], 1.0, kv[:], op0=Alu.mult, op1=Alu.mult,
                    accum_out=gf[:],
                )

                # --- h_attn^T = (rstdq_bc * gf) * Q^T * gate ---
                at = ap_.tile([R, C], bf16, tag="at")
                nc.vector.scalar_tensor_tensor(
                    at[:], bc_ps[0:R, :], gf[:], qk_s[0:R, :], op0=Alu.mult, op1=Alu.mult
                )
                ha = ap_.tile([R + 1, C], bf16, tag="ha")
                nc.vector.tensor_tensor(ha[0:R, :], at[:], gate[:], op=Alu.mult)
                nc.gpsimd.memset(ha[R : R + 1, :], 1.0)
                return ha

            def mix_phase(b, xb, ha):
                # --- upmix (+ residual via identity matmul) + LN, per half of 8 chunks ---
                sums = sp.tile([128, NCH], f32, tag="sums")
                sums2 = sp.tile([128, NCH], f32, tag="sums2")
                for g in range(2):
                    # process 8 chunks as pairs so only 2 PSUM up-banks are
                    # alive at a time (pool has 3)
                    vts = []
                    sqd_d = sqp.tile([128, C], bf16, tag="sqdd")
                    sqd_p = sqp.tile([128, C], bf16, tag="sqdp")
                    for h in range(4):
                        ups = []
                        for j in (2 * h, 2 * h + 1):
                            k = g * 8 + j
                            up_ps = ps_up.tile([128, C], f32, tag="up")
                            nc.tensor.matmul(
                                up_ps[:], wut[:, k * 128 : (k + 1) * 128], ha[:],
                                start=True, stop=False,
                            )
                            ups.append(up_ps)
                        for i, j in enumerate((2 * h, 2 * h + 1)):
                            k = g * 8 + j
                            nc.tensor.matmul(
                                ups[i][:], id128[:], xb[:, k * C : (k + 1) * C],
                                start=False, stop=True,
                            )
                        # v materialize (bf16) + row sums; then squares + row sums
                        for i, j in enumerate((2 * h, 2 * h + 1)):
                            k = g * 8 + j
                            vt = vp.tile([128, C], bf16, tag="v")
                            if j in VT_DVE_J:
                                nc.vector.tensor_scalar(
                                    vt[:], ups[i][:], 1.0, 0.0, op0=Alu.mult,
                                    op1=Alu.add, accum_out=sums[:, k : k + 1],
                                )
                            else:
                                nc.scalar.activation(
                                    vt[:], ups[i][:], Act.Identity,
                                    accum_out=sums[:, k : k + 1],
                                )
                            if j in SQ_POOL_J:
                                sq_eng, sqd = nc.gpsimd, sqd_p
                            else:
                                sq_eng, sqd = nc.vector, sqd_d
                            sq_eng.scalar_tensor_tensor(
                                sqd[:], vt[:], 1.0, vt[:], op0=Alu.mult, op1=Alu.mult,
                                accum_out=sums2[:, k : k + 1],
                            )
                            vts.append(vt)

                    # half-batch LN stats: var = E[v^2] - mu^2; rstd = 1/sqrt(var+eps)
                    gsl = slice(g * 8, g * 8 + 8)
                    nmu2 = sp.tile([128, 8], f32, tag="nmu2")
                    nc.vector.scalar_tensor_tensor(
                        nmu2[:], sums[:, gsl], -1.0 / (C * C), sums[:, gsl],
                        op0=Alu.mult, op1=Alu.mult,
                    )
                    var = sp.tile([128, 8], f32, tag="var")
                    nc.vector.scalar_tensor_tensor(
                        var[:], sums2[:, gsl], 1.0 / C, nmu2[:], op0=Alu.mult, op1=Alu.add
                    )
                    sd = sp.tile([128, 8], f32, tag="sd")
                    nc.scalar.activation(sd[:], var[:], Act.Sqrt, bias=epsl[:])
                    r0 = sp.tile([128, 8], f32, tag="r0")
                    nc.vector.reciprocal(r0[:], sd[:])
                    nmr = sp.tile([128, 8], f32, tag="nmr")
                    nc.vector.scalar_tensor_tensor(
                        nmr[:], sums[:, gsl], -1.0 / C, r0[:], op0=Alu.mult, op1=Alu.mult
                    )

                    # finalize y = (v - mu) * rstd [* gamma + beta], store half
                    yg = yp.tile([128, 8 * C], bf16, tag="y")
                    for j in range(8):
                        ys = yg[:, j * C : (j + 1) * C]
                        if j in YS_ACT_J:
                            nc.scalar.activation(
                                ys, vts[j][:], Act.Identity,
                                bias=nmr[:, j : j + 1], scale=r0[:, j : j + 1],
                            )
                        else:
                            eng = nc.vector if j in YS_DVE_J else nc.gpsimd
                            eng.tensor_scalar(
                                ys, vts[j][:], r0[:, j : j + 1], nmr[:, j : j + 1],
                                op0=Alu.mult, op1=Alu.add,
                            )
                        if not trivial_affine:
                            nc.vector.tensor_tensor(ys, ys, gam[:], op=Alu.mult)
                            nc.gpsimd.tensor_tensor(ys, ys, bet[:], op=Alu.add)
                    nc.sync.dma_start(
                        out_ap[b, g * 1024 : (g + 1) * 1024, :].rearrange(
                            "(n p j) c -> p n j c", p=128, j=2
                        ),
                        yg[:].rearrange("p (n j c) -> p n j c", n=4, j=2),
                    )

            def emit_all():
                # two-stage software pipeline: emit A(b+2) before B(b+1) so the
                # in-order PE stream runs batch b+2's matmuls while batch b+1's
                # serial attention chain finishes on ACT/DVE.
                xbs, has = {}, {}

                def A(b):
                    if b + 1 < BPC:
                        xbs[b + 1] = load_x(b + 1)
                    has[b] = attn_phase(b, xbs[b])

                xbs[0] = load_x(0)
                A(0)
                if BPC > 1:
                    A(1)
                for b in range(BPC):
                    mix_phase(b, xbs.pop(b), has.pop(b))
                    if b + 2 < BPC:
                        A(b + 2)

            if repeat == 1:
                emit_all()
            else:
                with tc.For_i(0, repeat, 1):
                    emit_all()

    return _split_waits(nc)


def prep_inputs(x, Wd, bd, Wq, Wk, Wv, Wg, bg, Wu, bu, gamma, beta, alpha):
    x = np.asarray(x, dtype=np.float32)
    Wd = np.asarray(Wd, np.float32); bd = np.asarray(bd, np.float32)
    Wq = np.asarray(Wq, np.float32); Wk = np.asarray(Wk, np.float32)
    Wv = np.asarray(Wv, np.float32); Wg = np.asarray(Wg, np.float32)
    bg = np.asarray(bg, np.float32)
    Wu = np.asarray(Wu, np.float32); bu = np.asarray(bu, np.float32)
    gamma = np.asarray(gamma, np.float32); beta = np.asarray(beta, np.float32)
    alpha = np.float32(np.asarray(alpha))

    trivial = bool(np.all(gamma == 1.0) and np.all(beta == 0.0))

    # row pairing: l = n*256 + 2p + j  <->  chunk k = 2n + j, partition p
    # wdt[p, k*R + r] = Wd.T[l(p,k), r]
    wdt = np.ascontiguousarray(
        Wd.T.reshape(NCH // 2, 128, 2, R).transpose(1, 0, 2, 3).reshape(128, NCH * R)
    )
    bdc = np.ascontiguousarray(bd[:, None])                # (R, 1)
    wqk = np.zeros((R + 1, 128), np.float32)
    wqk[0:R, 0:R] = Wq.T
    wqk[0:R, R:128] = Wk.T
    wvg = np.zeros((R + 1, 128), np.float32)
    wvg[0:R, 0:R] = Wv.T
    wvg[0:R, R:128] = Wg.T
    wvg[R, R:128] = bg
    # wut[:, k*128 + p] = alpha * [Wu.T; bu][:, l(p,k)]
    wut_full = np.zeros((R + 1, L), np.float32)
    wut_full[0:R, :] = alpha * Wu.T
    wut_full[R, :] = alpha * bu
    wut = np.ascontiguousarray(
        wut_full.reshape(R + 1, NCH // 2, 128, 2).transpose(0, 1, 3, 2).reshape(R + 1, NCH * 128)
    )
    selm = np.zeros((128, 2), np.float32)
    selm[0:R, 0] = 1.0
    selm[R:128, 1] = 1.0
    sel2 = np.zeros((2, 128), np.float32)
    sel2[0, 0:R] = 1.0
    sel2[1, R:128] = 1.0

    common = dict(
        wdt=wdt.astype(ml_dtypes.bfloat16),
        bdc=bdc,
        wqk=wqk.astype(ml_dtypes.bfloat16),
        wvg=wvg.astype(ml_dtypes.bfloat16),
        wut=wut.astype(ml_dtypes.bfloat16),
        id128=np.eye(128, dtype=ml_dtypes.bfloat16),
        selm=selm.astype(ml_dtypes.bfloat16),
        sel2=sel2.astype(ml_dtypes.bfloat16),
        epsn=np.full((2, 1), EPS_NORM_SQ, np.float32),
        epsl=np.full((128, 1), EPS_LN, np.float32),
    )
    if not trivial:
        common["gamb"] = np.ascontiguousarray(np.tile(gamma[None, :], (128, 1)))
        common["betb"] = np.ascontiguousarray(np.tile(beta[None, :], (128, 1)))

    xbf = x.astype(ml_dtypes.bfloat16)
    in_maps = []
    for c in range(N_CORES):
        m = dict(common)
        m["x"] = np.ascontiguousarray(xbf[c * BPC : (c + 1) * BPC])
        in_maps.append(m)
    return in_maps, trivial


_nc_cache = {}


def kernel(**inputs) -> np.ndarray:
    in_maps, trivial = prep_inputs(**inputs)
    if trivial not in _nc_cache:
        _nc_cache[trivial] = build(trivial)
    nc = _nc_cache[trivial]
    res = bass_utils.run_bass_kernel_spmd(nc, in_maps, core_ids=list(range(N_CORES)))
    out = np.concatenate([res.results[c]["out"] for c in range(N_CORES)], axis=0)
    return out.astype(np.float32)
